# revision 3
# baseline (speedup 1.0000x reference)
"""GQA causal attention (B=2, S=2048, HID=2048, H=32, HKV=8, D=128) on 8 TRN2
NeuronCores.

Sharding: tensor-parallel over heads. Core c owns kv head c and its 4 GQA
query heads (4c..4c+3): Wq column-shard [2048,512], Wk/Wv column-shard
[2048,128], plus a column shard of Wo [4096,256] (so no per-core dynamic
slicing is needed after the all-gather — every core computes the full-sequence
output for 1/8 of the hidden columns).

Device pipeline (bf16 compute, fp32 PSUM accumulation):
  1. Feature-major projections: Q^T/K^T/V^T = W^T h^T, with h^T streamed.
  2. RoPE as  x*cos_dup + (R @ x)*sin_dup  where R is the rotate-half
     permutation (+-1 matrix) applied via a tiny TensorE matmul — avoids
     illegal cross-partition-base DVE operands.
  3. Transposed flash attention: S^T[k,q] tiles = K^T_chunk.T @ Q^T, exp on
     ScalarE without max subtraction (scores bounded ~ +-5), causal masking by
     0/1 multiply on diagonal tiles, denominator via ones-matmul accumulated
     in PSUM, P^T @ nothing-transposed: out^T accumulates with V tiles as the
     stationary operand.
  4. AllGather of normalized attn-out^T (bf16, 4MB/rank) across the 8 cores.
  5. o_proj: out^T[hid_slice, s] = Wo_shard^T-chunks @ attn_full^T, fp32 out.
Host reassembles the 8 hidden-column slices and transposes back.
"""

import os

import numpy as np
import ml_dtypes

from concourse import bacc, mybir
import concourse.tile as tile
from concourse.bass_utils import run_bass_kernel_spmd

N_CORES = 8
B, S, HID = 2, 2048, 2048
H, HKV, D = 32, 8, 128
QH = H // HKV          # q heads per core
SG = B * S             # 4096 global sequence
NSC = SG // 512        # 8 s-chunks of 512
NKT = HID // 128       # 16 hid k-tiles
NFT = (H * D) // 128   # 32 o_proj contraction tiles

BF = mybir.dt.bfloat16
F32 = mybir.dt.float32
AF = mybir.ActivationFunctionType

_CACHE = {}
LAST_EXEC_NS = None


def _build():
    nc = bacc.Bacc("TRN2", num_devices=N_CORES)

    hT_e = nc.declare_dram_parameter("hT", [HID, SG], BF, isOutput=False)
    wq_e = nc.declare_dram_parameter("wq", [HID, QH * D], BF, isOutput=False)
    wk_e = nc.declare_dram_parameter("wk", [HID, D], BF, isOutput=False)
    wv_e = nc.declare_dram_parameter("wv", [HID, D], BF, isOutput=False)
    wo_e = nc.declare_dram_parameter("wo", [H * D, HID // N_CORES], BF, isOutput=False)
    cd_e = nc.declare_dram_parameter("c_dup", [D, SG], BF, isOutput=False)
    sd_e = nc.declare_dram_parameter("s_dup", [D, SG], BF, isOutput=False)
    rT_e = nc.declare_dram_parameter("rT", [D, D], BF, isOutput=False)
    id_e = nc.declare_dram_parameter("ident", [D, D], BF, isOutput=False)
    mk_e = nc.declare_dram_parameter("masks", [128, 4 * 512], BF, isOutput=False)
    outT_e = nc.declare_dram_parameter(
        "outT", [HID // N_CORES, SG], F32, isOutput=True
    )

    with tile.TileContext(nc) as tc:
        with (
            tc.tile_pool(name="cst", bufs=1) as cst,
            tc.tile_pool(name="sb", bufs=2) as sb,
            tc.tile_pool(name="ps", bufs=3, space="PSUM") as ps,
            tc.tile_pool(name="psacc", bufs=2, space="PSUM") as psacc,
            tc.tile_pool(name="dram", bufs=1, space="DRAM") as dram,
        ):
            # ---- persistent constants / intermediates ----
            wo_sb = cst.tile([128, NFT, HID // N_CORES], BF, tag="wo_sb")
            nc.sync.dma_start(
                wo_sb[:], wo_e[:].rearrange("(ft p) f -> p ft f", p=128)
            )
            masks = cst.tile([128, 4 * 512], BF, tag="masks")
            nc.sync.dma_start(masks[:], mk_e[:])
            ones = cst.tile([128, 1], BF, tag="ones")
            nc.gpsimd.memset(ones[:], 1.0)

            qr = cst.tile([128, QH * SG], BF, tag="qr")
            kr = cst.tile([128, SG], BF, tag="kr")
            v_seq = cst.tile([128, SG], BF, tag="v_seq")

            agin = dram.tile([QH * D, SG], BF)
            agout = dram.tile([H * D, SG], BF, addr_space="Shared")

            # ---- phase 1: projections + rope + V transpose ----
            with tc.tile_pool(name="p1", bufs=1) as p1, \
                 tc.tile_pool(name="htp", bufs=2) as htp:
                wq_sb = p1.tile([128, NKT, QH * D], BF, tag="wq_sb")
                nc.sync.dma_start(
                    wq_sb[:], wq_e[:].rearrange("(kt p) f -> p kt f", p=128)
                )
                wk_sb = p1.tile([128, NKT, D], BF, tag="wk_sb")
                nc.sync.dma_start(
                    wk_sb[:], wk_e[:].rearrange("(kt p) f -> p kt f", p=128)
                )
                wv_sb = p1.tile([128, NKT, D], BF, tag="wv_sb")
                nc.sync.dma_start(
                    wv_sb[:], wv_e[:].rearrange("(kt p) f -> p kt f", p=128)
                )
                c_d = p1.tile([D, SG], BF, tag="c_d")
                nc.sync.dma_start(c_d[:], cd_e[:])
                s_d = p1.tile([D, SG], BF, tag="s_d")
                nc.sync.dma_start(s_d[:], sd_e[:])
                rT = p1.tile([D, D], BF, tag="rT")
                nc.sync.dma_start(rT[:], rT_e[:])
                ident = p1.tile([D, D], BF, tag="ident")
                nc.sync.dma_start(ident[:], id_e[:])

                with nc.named_scope("proj"):
                    for sc in range(NSC):
                        ht = htp.tile([128, NKT, 512], BF, tag="ht")
                        nc.sync.dma_start(
                            ht[:],
                            hT_e[:, sc * 512 : (sc + 1) * 512].rearrange(
                                "(kt p) s -> p kt s", p=128
                            ),
                        )
                        for ft in range(QH + 2):  # 0..3 q heads, 4 k, 5 v
                            acc = ps.tile([128, 512], F32, tag="mm")
                            for kt in range(NKT):
                                if ft < QH:
                                    lhsT = wq_sb[:, kt, ft * D : (ft + 1) * D]
                                elif ft == QH:
                                    lhsT = wk_sb[:, kt, :]
                                else:
                                    lhsT = wv_sb[:, kt, :]
                                nc.tensor.matmul(
                                    acc[:], lhsT, ht[:, kt, :],
                                    start=(kt == 0), stop=(kt == NKT - 1),
                                )
                            xb = sb.tile([128, 512], BF, tag="xb", bufs=3)
                            nc.scalar.activation(xb[:], acc[:], AF.Copy)
                            if ft < QH + 1:  # rope for q heads and k
                                rot = ps.tile([128, 512], F32, tag="mm")
                                nc.tensor.matmul(rot[:], rT[:], xb[:])
                                if ft < QH:
                                    dest = qr[
                                        :, ft * SG + sc * 512 : ft * SG + sc * 512 + 512
                                    ]
                                else:
                                    dest = kr[:, sc * 512 : sc * 512 + 512]
                                cs = c_d[:, sc * 512 : (sc + 1) * 512]
                                ss = s_d[:, sc * 512 : (sc + 1) * 512]
                                nc.vector.tensor_mul(dest, xb[:], cs)
                                rtmp = sb.tile([128, 512], BF, tag="rtmp")
                                nc.vector.tensor_mul(rtmp[:], rot[:], ss)
                                nc.vector.tensor_add(dest, dest, rtmp[:])
                            else:  # v: transpose to seq-major
                                for j in range(4):
                                    tp = ps.tile([128, 128], BF, tag="mm")
                                    nc.tensor.transpose(
                                        tp[:], xb[:, j * 128 : (j + 1) * 128], ident[:]
                                    )
                                    g = sc * 4 + j
                                    nc.vector.tensor_copy(
                                        v_seq[:, g * 128 : (g + 1) * 128], tp[:]
                                    )

            # ---- phase 2: attention ----
            with nc.named_scope("attn"):
                for b in range(B):
                    for h in range(QH):
                        for qc in range(4):  # q-chunk of 512 within batch
                            nkt = 4 * qc + 4
                            acc = psacc.tile([128, 512], F32, tag="acc")
                            den = psacc.tile([1, 512], F32, tag="den")
                            qs = h * SG + b * S + qc * 512
                            for kt in range(nkt):
                                s_ps = ps.tile([128, 512], F32, tag="mm")
                                nc.tensor.matmul(
                                    s_ps[:],
                                    kr[:, b * S + kt * 128 : b * S + (kt + 1) * 128],
                                    qr[:, qs : qs + 512],
                                )
                                pT = sb.tile([128, 512], BF, tag="pT", bufs=3)
                                nc.scalar.activation(pT[:], s_ps[:], AF.Exp)
                                j = kt - 4 * qc
                                if j >= 0:
                                    nc.vector.tensor_mul(
                                        pT[:], pT[:],
                                        masks[:, j * 512 : (j + 1) * 512],
                                    )
                                nc.tensor.matmul(
                                    den[:], ones[:], pT[:],
                                    start=(kt == 0), stop=(kt == nkt - 1),
                                )
                                g = b * 16 + kt
                                nc.tensor.matmul(
                                    acc[:], v_seq[:, g * 128 : (g + 1) * 128], pT[:],
                                    start=(kt == 0), stop=(kt == nkt - 1),
                                )
                            recip = sb.tile([1, 512], F32, tag="recip")
                            nc.vector.reciprocal(recip[:], den[:])
                            rb = sb.tile([128, 512], F32, tag="rb")
                            nc.gpsimd.partition_broadcast(rb[:], recip[:])
                            ao = sb.tile([128, 512], BF, tag="ao", bufs=3)
                            nc.vector.tensor_mul(ao[:], acc[:], rb[:])
                            nc.sync.dma_start(
                                agin[
                                    h * D : (h + 1) * D,
                                    b * S + qc * 512 : b * S + (qc + 1) * 512,
                                ],
                                ao[:],
                            )

            # ---- phase 3: all-gather ----
            nc.gpsimd.collective_compute(
                "AllGather",
                mybir.AluOpType.bypass,
                replica_groups=[list(range(N_CORES))],
                ins=[agin.opt()],
                outs=[agout.opt()],
            )

            # ---- phase 4: o_proj on hidden-column slice ----
            with nc.named_scope("oproj"), tc.tile_pool(name="agp", bufs=2) as agp:
                for sc in range(NSC):
                    ag = agp.tile([128, NFT, 512], BF, tag="ag")
                    nc.sync.dma_start(
                        ag[:],
                        agout[:, sc * 512 : (sc + 1) * 512].rearrange(
                            "(ft p) s -> p ft s", p=128
                        ),
                    )
                    for hid_t in range(2):
                        o_ps = ps.tile([128, 512], F32, tag="mm")
                        for ft in range(NFT):
                            nc.tensor.matmul(
                                o_ps[:],
                                wo_sb[:, ft, hid_t * 128 : (hid_t + 1) * 128],
                                ag[:, ft, :],
                                start=(ft == 0), stop=(ft == NFT - 1),
                            )
                        ob = sb.tile([128, 512], F32, tag="ob")
                        nc.scalar.activation(ob[:], o_ps[:], AF.Copy)
                        nc.sync.dma_start(
                            outT_e[
                                hid_t * 128 : (hid_t + 1) * 128,
                                sc * 512 : (sc + 1) * 512,
                            ],
                            ob[:],
                        )

    nc.compile()
    return nc


def _prep(hidden_states, sin_table, cos_table, Wq, Wk, Wv, Wo):
    bf = ml_dtypes.bfloat16
    flat = np.asarray(hidden_states, np.float32).reshape(SG, HID)
    hT = np.ascontiguousarray(flat.T).astype(bf)

    cosT = np.asarray(cos_table, np.float32)[:, :64].T  # [64, S]
    sinT = np.asarray(sin_table, np.float32)[:, :64].T
    c_dup = np.tile(np.concatenate([cosT, cosT], 0), (1, B)).astype(bf)
    s_dup = np.tile(np.concatenate([sinT, sinT], 0), (1, B)).astype(bf)

    R = np.zeros((D, D), np.float32)
    for i in range(64):
        R[i, i + 64] = -1.0
        R[i + 64, i] = 1.0
    rT = np.ascontiguousarray(R.T).astype(bf)
    ident = np.eye(D, dtype=np.float32).astype(bf)

    kk = np.arange(128)[:, None]
    qq = np.arange(512)[None, :]
    masks = np.concatenate(
        [(j * 128 + kk <= qq).astype(np.float32) for j in range(4)], axis=1
    ).astype(bf)

    scale = np.float32(1.0 / np.sqrt(D))
    Wq = np.asarray(Wq, np.float32) * scale
    Wk = np.asarray(Wk, np.float32)
    Wv = np.asarray(Wv, np.float32)
    Wo = np.asarray(Wo, np.float32)

    in_maps = []
    for c in range(N_CORES):
        in_maps.append(
            {
                "hT": hT,
                "wq": np.ascontiguousarray(Wq[:, c * 512 : (c + 1) * 512]).astype(bf),
                "wk": np.ascontiguousarray(Wk[:, c * D : (c + 1) * D]).astype(bf),
                "wv": np.ascontiguousarray(Wv[:, c * D : (c + 1) * D]).astype(bf),
                "wo": np.ascontiguousarray(Wo[:, c * 256 : (c + 1) * 256]).astype(bf),
                "c_dup": c_dup,
                "s_dup": s_dup,
                "rT": rT,
                "ident": ident,
                "masks": masks,
            }
        )
    return in_maps


def kernel(**inputs) -> np.ndarray:
    global LAST_EXEC_NS
    if "nc" not in _CACHE:
        _CACHE["nc"] = _build()
    nc = _CACHE["nc"]

    in_maps = _prep(**inputs)
    res = run_bass_kernel_spmd(
        nc,
        in_maps,
        core_ids=list(range(N_CORES)),
        trace=bool(os.environ.get("BASS_TRACE")),
    )
    LAST_EXEC_NS = res.exec_time_ns

    outT = np.concatenate(
        [np.asarray(res.results[c]["outT"], np.float32) for c in range(N_CORES)],
        axis=0,
    )  # [HID, SG]
    return np.ascontiguousarray(outT.T).reshape(B, S, HID)


# revision 5
# speedup vs baseline: 1.1557x; 1.1557x over previous
"""GQA causal attention (B=2, S=2048, HID=2048, H=32, HKV=8, D=128) on 8 TRN2
NeuronCores.

Sharding: tensor-parallel over heads for QKV+attention (core c owns kv head c
and q heads 4c..4c+3), then an AllToAll switches to sequence-parallel for
o_proj (core c computes the full hidden dim for global s-chunk c). The A2A
moves 8x less data than an AllGather and needs no per-core dynamic slicing.
It is split into two collectives (head pairs) so comm overlaps attention
compute of the remaining heads and the first half of o_proj.

Device pipeline (bf16 compute, fp32 PSUM accumulation):
  1. Feature-major projections: Q^T/K^T/V^T = W^T h^T, h^T streamed.
  2. RoPE as  x*cos_dup + (R @ x)*sin_dup  with R = rotate-half permutation
     applied via a small TensorE matmul (cross-partition DVE ops are illegal).
  3. Transposed flash attention: S^T[k,q] = K^T_chunk.T @ Q^T chunk, exp on
     ScalarE without max subtraction (scores bounded), causal 0/1 mask on
     diagonal tiles, denominator via ones-matmul, out^T += V_tile.T @ P^T.
     Score matmuls are emitted 2 tiles ahead so the TensorE never waits on
     ScalarE's exp.
  4. Two AllToAlls (heads 0-1, then 2-3) exchange attn-out^T blocks.
  5. o_proj: out^T[hid, my_s_chunk] accumulated over all 32 feature tiles
     (Wo host-permuted into A2A block order, streamed), fp32 out.
Host reassembles the 8 sequence chunks and transposes back.
"""

import os

import numpy as np
import ml_dtypes

from concourse import bacc, mybir
import concourse.tile as tile
from concourse.bass_utils import run_bass_kernel_spmd

N_CORES = 8
B, S, HID = 2, 2048, 2048
H, HKV, D = 32, 8, 128
QH = H // HKV          # q heads per core
SG = B * S             # 4096 global sequence
NSC = SG // 512        # 8 s-chunks of 512
NKT = HID // 128       # 16 hid k-tiles
NFT = (H * D) // 128   # 32 o_proj contraction tiles

BF = mybir.dt.bfloat16
F32 = mybir.dt.float32
AF = mybir.ActivationFunctionType

_CACHE = {}
LAST_EXEC_NS = None


def _build():
    nc = bacc.Bacc("TRN2", num_devices=N_CORES)

    hT_e = nc.declare_dram_parameter("hT", [HID, SG], BF, isOutput=False)
    wq_e = nc.declare_dram_parameter("wq", [HID, QH * D], BF, isOutput=False)
    wk_e = nc.declare_dram_parameter("wk", [HID, D], BF, isOutput=False)
    wv_e = nc.declare_dram_parameter("wv", [HID, D], BF, isOutput=False)
    # Wo rows pre-permuted on host into (a2a1 blocks, a2a2 blocks) order.
    wo_e = nc.declare_dram_parameter("wo", [H * D, HID], BF, isOutput=False)
    cd_e = nc.declare_dram_parameter("c_dup", [D, SG], BF, isOutput=False)
    sd_e = nc.declare_dram_parameter("s_dup", [D, SG], BF, isOutput=False)
    rT_e = nc.declare_dram_parameter("rT", [D, D], BF, isOutput=False)
    id_e = nc.declare_dram_parameter("ident", [D, D], BF, isOutput=False)
    mk_e = nc.declare_dram_parameter("masks", [128, 4 * 512], BF, isOutput=False)
    outT_e = nc.declare_dram_parameter("outT", [HID, 512], F32, isOutput=True)

    with tile.TileContext(nc) as tc:
        with (
            tc.tile_pool(name="cst", bufs=1) as cst,
            tc.tile_pool(name="sb", bufs=2) as sb,
            tc.tile_pool(name="ps", bufs=3, space="PSUM") as ps,
            tc.tile_pool(name="psacc", bufs=2, space="PSUM") as psacc,
            tc.tile_pool(name="dram", bufs=1, space="DRAM") as dram,
        ):
            masks = cst.tile([128, 4 * 512], BF, tag="masks")
            nc.sync.dma_start(masks[:], mk_e[:])
            ones = cst.tile([128, 1], BF, tag="ones")
            nc.gpsimd.memset(ones[:], 1.0)

            qr = cst.tile([128, QH * SG], BF, tag="qr")
            kr = cst.tile([128, SG], BF, tag="kr")
            v_seq = cst.tile([128, SG], BF, tag="v_seq")

            # A2A bounce buffers: shard j = rows [j*256, (j+1)*256) =
            # (2 heads x 128d, s-chunk j's 512 cols).
            a2a_in = [dram.tile([8 * 256, 512], BF, name=f"a2ain{i}") for i in (0, 1)]
            a2a_out = [
                dram.tile([8 * 256, 512], BF, name=f"a2aout{i}") for i in (0, 1)
            ]

            # ---- phase 1: projections + rope + V transpose ----
            with tc.tile_pool(name="p1", bufs=1) as p1, \
                 tc.tile_pool(name="htp", bufs=2) as htp:
                wq_sb = p1.tile([128, NKT, QH * D], BF, tag="wq_sb")
                nc.sync.dma_start(
                    wq_sb[:], wq_e[:].rearrange("(kt p) f -> p kt f", p=128)
                )
                wk_sb = p1.tile([128, NKT, D], BF, tag="wk_sb")
                nc.sync.dma_start(
                    wk_sb[:], wk_e[:].rearrange("(kt p) f -> p kt f", p=128)
                )
                wv_sb = p1.tile([128, NKT, D], BF, tag="wv_sb")
                nc.sync.dma_start(
                    wv_sb[:], wv_e[:].rearrange("(kt p) f -> p kt f", p=128)
                )
                c_d = p1.tile([D, SG], BF, tag="c_d")
                nc.sync.dma_start(c_d[:], cd_e[:])
                s_d = p1.tile([D, SG], BF, tag="s_d")
                nc.sync.dma_start(s_d[:], sd_e[:])
                rT = p1.tile([D, D], BF, tag="rT")
                nc.sync.dma_start(rT[:], rT_e[:])
                ident = p1.tile([D, D], BF, tag="ident")
                nc.sync.dma_start(ident[:], id_e[:])

                with nc.named_scope("proj"):
                    for sc in range(NSC):
                        ht = htp.tile([128, NKT, 512], BF, tag="ht")
                        nc.sync.dma_start(
                            ht[:],
                            hT_e[:, sc * 512 : (sc + 1) * 512].rearrange(
                                "(kt p) s -> p kt s", p=128
                            ),
                        )
                        for ft in range(QH + 2):  # 0..3 q heads, 4 k, 5 v
                            acc = ps.tile([128, 512], F32, tag="mm")
                            for kt in range(NKT):
                                if ft < QH:
                                    lhsT = wq_sb[:, kt, ft * D : (ft + 1) * D]
                                elif ft == QH:
                                    lhsT = wk_sb[:, kt, :]
                                else:
                                    lhsT = wv_sb[:, kt, :]
                                nc.tensor.matmul(
                                    acc[:], lhsT, ht[:, kt, :],
                                    start=(kt == 0), stop=(kt == NKT - 1),
                                )
                            xb = sb.tile([128, 512], BF, tag="xb", bufs=3)
                            nc.scalar.activation(xb[:], acc[:], AF.Copy)
                            if ft < QH + 1:  # rope for q heads and k
                                rot = ps.tile([128, 512], F32, tag="mm")
                                nc.tensor.matmul(rot[:], rT[:], xb[:])
                                if ft < QH:
                                    dest = qr[
                                        :, ft * SG + sc * 512 : ft * SG + sc * 512 + 512
                                    ]
                                else:
                                    dest = kr[:, sc * 512 : sc * 512 + 512]
                                cs = c_d[:, sc * 512 : (sc + 1) * 512]
                                ss = s_d[:, sc * 512 : (sc + 1) * 512]
                                nc.vector.tensor_mul(dest, xb[:], cs)
                                rtmp = sb.tile([128, 512], BF, tag="rtmp")
                                nc.vector.tensor_mul(rtmp[:], rot[:], ss)
                                nc.vector.tensor_add(dest, dest, rtmp[:])
                            else:  # v: transpose to seq-major
                                for j in range(4):
                                    tp = ps.tile([128, 128], BF, tag="mm")
                                    nc.tensor.transpose(
                                        tp[:], xb[:, j * 128 : (j + 1) * 128], ident[:]
                                    )
                                    g = sc * 4 + j
                                    nc.vector.tensor_copy(
                                        v_seq[:, g * 128 : (g + 1) * 128], tp[:]
                                    )

            # ---- phase 2: attention (h outer so A2A can fire per head-pair)
            def attn_head(h, b, qc):
                nkt = 4 * qc + 4
                acc = psacc.tile([128, 512], F32, tag="acc")
                den = psacc.tile([1, 512], F32, tag="den")
                qs = h * SG + b * S + qc * 512

                def score(kt):
                    s_ps = ps.tile([128, 512], F32, tag="mm", name=f"s_{h}_{b}_{qc}_{kt}")
                    nc.tensor.matmul(
                        s_ps[:],
                        kr[:, b * S + kt * 128 : b * S + (kt + 1) * 128],
                        qr[:, qs : qs + 512],
                    )
                    return s_ps

                pipe = [score(0)]
                if nkt > 1:
                    pipe.append(score(1))
                for kt in range(nkt):
                    if kt + 2 < nkt:
                        pipe.append(score(kt + 2))
                    s_ps = pipe.pop(0)
                    pT = sb.tile([128, 512], BF, tag="pT", bufs=3)
                    nc.scalar.activation(pT[:], s_ps[:], AF.Exp)
                    j = kt - 4 * qc
                    if j >= 0:
                        nc.vector.tensor_mul(
                            pT[:], pT[:], masks[:, j * 512 : (j + 1) * 512]
                        )
                    nc.tensor.matmul(
                        den[:], ones[:], pT[:],
                        start=(kt == 0), stop=(kt == nkt - 1),
                    )
                    g = b * 16 + kt
                    nc.tensor.matmul(
                        acc[:], v_seq[:, g * 128 : (g + 1) * 128], pT[:],
                        start=(kt == 0), stop=(kt == nkt - 1),
                    )
                recip = sb.tile([1, 512], F32, tag="recip")
                nc.vector.reciprocal(recip[:], den[:])
                rb = sb.tile([128, 512], F32, tag="rb")
                nc.gpsimd.partition_broadcast(rb[:], recip[:])
                ao = sb.tile([128, 512], BF, tag="ao", bufs=3)
                nc.vector.tensor_mul(ao[:], acc[:], rb[:])
                half, hh = divmod(h, 2)
                sc = b * 4 + qc
                nc.sync.dma_start(
                    a2a_in[half][sc * 256 + hh * 128 : sc * 256 + (hh + 1) * 128, :],
                    ao[:],
                )

            with nc.named_scope("attn"):
                for half in range(2):
                    for h in (2 * half, 2 * half + 1):
                        for b in range(B):
                            for qc in range(4):
                                attn_head(h, b, qc)
                    nc.gpsimd.collective_compute(
                        "AllToAll",
                        mybir.AluOpType.bypass,
                        replica_groups=[list(range(N_CORES))],
                        ins=[a2a_in[half].opt()],
                        outs=[a2a_out[half].opt()],
                    )

            # ---- phase 4: o_proj for my s-chunk, all hidden columns ----
            with nc.named_scope("oproj"), \
                 tc.tile_pool(name="agp", bufs=1) as agp, \
                 tc.tile_pool(name="wop", bufs=2) as wop:
                ag = []
                for half in range(2):
                    agt = agp.tile([128, 16, 512], BF, tag=f"ag{half}")
                    nc.sync.dma_start(
                        agt[:],
                        a2a_out[half][:].rearrange("(ft p) s -> p ft s", p=128),
                    )
                    ag.append(agt)
                for hid_t in range(NKT):  # 16 tiles of 128 hidden cols
                    wo_t = wop.tile([128, NFT, 128], BF, tag="wo_t")
                    nc.sync.dma_start(
                        wo_t[:],
                        wo_e[:, hid_t * 128 : (hid_t + 1) * 128].rearrange(
                            "(ft p) c -> p ft c", p=128
                        ),
                    )
                    o_ps = ps.tile([128, 512], F32, tag="mm")
                    for half in range(2):
                        for ft in range(16):
                            nc.tensor.matmul(
                                o_ps[:],
                                wo_t[:, half * 16 + ft, :],
                                ag[half][:, ft, :],
                                start=(half == 0 and ft == 0),
                                stop=(half == 1 and ft == 15),
                            )
                    ob = sb.tile([128, 512], F32, tag="ob")
                    nc.scalar.activation(ob[:], o_ps[:], AF.Copy)
                    nc.sync.dma_start(
                        outT_e[hid_t * 128 : (hid_t + 1) * 128, :], ob[:]
                    )

    nc.compile()
    return nc


def _prep(hidden_states, sin_table, cos_table, Wq, Wk, Wv, Wo):
    bf = ml_dtypes.bfloat16
    flat = np.asarray(hidden_states, np.float32).reshape(SG, HID)
    hT = np.ascontiguousarray(flat.T).astype(bf)

    cosT = np.asarray(cos_table, np.float32)[:, :64].T  # [64, S]
    sinT = np.asarray(sin_table, np.float32)[:, :64].T
    c_dup = np.tile(np.concatenate([cosT, cosT], 0), (1, B)).astype(bf)
    s_dup = np.tile(np.concatenate([sinT, sinT], 0), (1, B)).astype(bf)

    R = np.zeros((D, D), np.float32)
    for i in range(64):
        R[i, i + 64] = -1.0
        R[i + 64, i] = 1.0
    rT = np.ascontiguousarray(R.T).astype(bf)
    ident = np.eye(D, dtype=np.float32).astype(bf)

    kk = np.arange(128)[:, None]
    qq = np.arange(512)[None, :]
    masks = np.concatenate(
        [(j * 128 + kk <= qq).astype(np.float32) for j in range(4)], axis=1
    ).astype(bf)

    scale = np.float32(1.0 / np.sqrt(D))
    Wq = np.asarray(Wq, np.float32) * scale
    Wk = np.asarray(Wk, np.float32)
    Wv = np.asarray(Wv, np.float32)
    Wo = np.asarray(Wo, np.float32)

    # Permute Wo rows into the order o_proj consumes the A2A output blocks:
    # a2a1 blocks: (r, h in {0,1}); a2a2 blocks: (r, h in {2,3}).
    Wo_b = Wo.reshape(H, D, HID)
    order = [4 * r + h for r in range(8) for h in (0, 1)] + [
        4 * r + h for r in range(8) for h in (2, 3)
    ]
    Wo_perm = np.ascontiguousarray(Wo_b[order].reshape(H * D, HID)).astype(bf)

    in_maps = []
    for c in range(N_CORES):
        in_maps.append(
            {
                "hT": hT,
                "wq": np.ascontiguousarray(Wq[:, c * 512 : (c + 1) * 512]).astype(bf),
                "wk": np.ascontiguousarray(Wk[:, c * D : (c + 1) * D]).astype(bf),
                "wv": np.ascontiguousarray(Wv[:, c * D : (c + 1) * D]).astype(bf),
                "wo": Wo_perm,
                "c_dup": c_dup,
                "s_dup": s_dup,
                "rT": rT,
                "ident": ident,
                "masks": masks,
            }
        )
    return in_maps


def kernel(**inputs) -> np.ndarray:
    global LAST_EXEC_NS
    if "nc" not in _CACHE:
        _CACHE["nc"] = _build()
    nc = _CACHE["nc"]

    in_maps = _prep(**inputs)
    res = run_bass_kernel_spmd(
        nc,
        in_maps,
        core_ids=list(range(N_CORES)),
        trace=bool(os.environ.get("BASS_TRACE")),
    )
    LAST_EXEC_NS = res.exec_time_ns

    outT = np.concatenate(
        [np.asarray(res.results[c]["outT"], np.float32) for c in range(N_CORES)],
        axis=1,
    )  # [HID, SG]
    return np.ascontiguousarray(outT.T).reshape(B, S, HID)


# revision 11
# speedup vs baseline: 1.2355x; 1.0690x over previous
"""GQA causal attention (B=2, S=2048, HID=2048, H=32, HKV=8, D=128) on 8 TRN2
NeuronCores.

Sharding: tensor-parallel over heads for QKV+attention (core c owns kv head c
and q heads 4c..4c+3), then an AllToAll switches to sequence-parallel for
o_proj (core c computes the full hidden dim for global s-chunk c). The A2A
moves 8x less data than an AllGather and needs no per-core dynamic slicing.
It is split into two collectives (head pairs) so comm overlaps attention
compute of the remaining heads and the first half of o_proj.

Device pipeline (bf16 compute, fp32 PSUM accumulation):
  1. Feature-major projections: Q^T/K^T/V^T = W^T h^T, h^T streamed.
  2. RoPE as  x*cos_dup + (R @ x)*sin_dup  with R = rotate-half permutation
     applied via a small TensorE matmul (cross-partition DVE ops are illegal).
  3. Transposed flash attention: S^T[k,q] = K^T_chunk.T @ Q^T chunk, exp on
     ScalarE without max subtraction (scores bounded), causal 0/1 mask on
     diagonal tiles, denominator via ones-matmul, out^T += V_tile.T @ P^T.
     Score matmuls are emitted 2 tiles ahead so the TensorE never waits on
     ScalarE's exp.
  4. Two AllToAlls (heads 0-1, then 2-3) exchange attn-out^T blocks.
  5. o_proj: out^T[hid, my_s_chunk] accumulated over all 32 feature tiles
     (Wo host-permuted into A2A block order, streamed), fp32 out.
Host reassembles the 8 sequence chunks and transposes back.
"""

import os

import numpy as np
import ml_dtypes

from concourse import bacc, mybir
import concourse.tile as tile
from concourse.bass_utils import run_bass_kernel_spmd

N_CORES = 8
B, S, HID = 2, 2048, 2048
H, HKV, D = 32, 8, 128
QH = H // HKV          # q heads per core
SG = B * S             # 4096 global sequence
NSC = SG // 512        # 8 s-chunks of 512
NKT = HID // 128       # 16 hid k-tiles
NFT = (H * D) // 128   # 32 o_proj contraction tiles

BF = mybir.dt.bfloat16
F32 = mybir.dt.float32
AF = mybir.ActivationFunctionType

_CACHE = {}
LAST_EXEC_NS = None


def _build():
    nc = bacc.Bacc("TRN2", num_devices=N_CORES)

    hT_e = nc.declare_dram_parameter("hT", [HID, SG], BF, isOutput=False)
    wq_e = nc.declare_dram_parameter("wq", [HID, QH * D], BF, isOutput=False)
    wk_e = nc.declare_dram_parameter("wk", [HID, D], BF, isOutput=False)
    wv_e = nc.declare_dram_parameter("wv", [HID, D], BF, isOutput=False)
    # Wo rows pre-permuted on host into (a2a1 blocks, a2a2 blocks) order.
    wo_e = nc.declare_dram_parameter("wo", [H * D, HID], BF, isOutput=False)
    # single lower-triangular mask tril[kk, q] = kk <= q
    cd_e = nc.declare_dram_parameter("c_dup", [D, SG], BF, isOutput=False)
    sd_e = nc.declare_dram_parameter("s_dup", [D, SG], BF, isOutput=False)
    rT_e = nc.declare_dram_parameter("rT", [D, D], BF, isOutput=False)
    id_e = nc.declare_dram_parameter("ident", [D, D], BF, isOutput=False)
    mk_e = nc.declare_dram_parameter("masks", [128, 512], BF, isOutput=False)
    outT_e = nc.declare_dram_parameter("outT", [HID, 512], F32, isOutput=True)

    with tile.TileContext(nc) as tc:
        with (
            tc.tile_pool(name="cst", bufs=1) as cst,
            tc.tile_pool(name="sb", bufs=2) as sb,
            tc.tile_pool(name="ps", bufs=3, space="PSUM") as ps,
            tc.tile_pool(name="psacc", bufs=2, space="PSUM") as psacc,
            tc.tile_pool(name="dram", bufs=1, space="DRAM") as dram,
        ):
            tril = cst.tile([128, 512], BF, tag="tril")
            nc.sync.dma_start(tril[:], mk_e[:])
            ones = cst.tile([128, 1], BF, tag="ones")
            nc.gpsimd.memset(ones[:], 1.0)

            qr = cst.tile([128, QH * SG], BF, tag="qr")
            kr = cst.tile([128, SG], BF, tag="kr")
            v_seq = cst.tile([128, SG], BF, tag="v_seq")

            # A2A bounce buffers: shard j = rows [j*256, (j+1)*256) =
            # (2 heads x 128d, s-chunk j's 512 cols).
            a2a_in = [dram.tile([8 * 256, 512], BF, name=f"a2ain{i}") for i in (0, 1)]
            a2a_out = [
                dram.tile([8 * 256, 512], BF, name=f"a2aout{i}") for i in (0, 1)
            ]

            # ---- phase 1: projections + rope + V transpose ----
            with tc.tile_pool(name="p1", bufs=1) as p1, \
                 tc.tile_pool(name="htp", bufs=2) as htp:
                wq_sb = p1.tile([128, NKT, QH * D], BF, tag="wq_sb")
                nc.sync.dma_start(
                    wq_sb[:], wq_e[:].rearrange("(kt p) f -> p kt f", p=128)
                )
                wk_sb = p1.tile([128, NKT, D], BF, tag="wk_sb")
                nc.sync.dma_start(
                    wk_sb[:], wk_e[:].rearrange("(kt p) f -> p kt f", p=128)
                )
                wv_sb = p1.tile([128, NKT, D], BF, tag="wv_sb")
                nc.sync.dma_start(
                    wv_sb[:], wv_e[:].rearrange("(kt p) f -> p kt f", p=128)
                )
                c_d = p1.tile([D, SG], BF, tag="c_d")
                nc.sync.dma_start(c_d[:], cd_e[:])
                s_d = p1.tile([D, SG], BF, tag="s_d")
                nc.sync.dma_start(s_d[:], sd_e[:])
                rT = p1.tile([D, D], BF, tag="rT")
                nc.sync.dma_start(rT[:], rT_e[:])
                ident = p1.tile([D, D], BF, tag="ident")
                nc.sync.dma_start(ident[:], id_e[:])

                with nc.named_scope("proj"):
                    for sc in range(NSC):
                        ht = htp.tile([128, NKT, 512], BF, tag="ht")
                        nc.sync.dma_start(
                            ht[:],
                            hT_e[:, sc * 512 : (sc + 1) * 512].rearrange(
                                "(kt p) s -> p kt s", p=128
                            ),
                        )
                        for ft in range(QH + 2):  # 0..3 q heads, 4 k, 5 v
                            acc = ps.tile([128, 512], F32, tag="mm")
                            for kt in range(NKT):
                                if ft < QH:
                                    lhsT = wq_sb[:, kt, ft * D : (ft + 1) * D]
                                elif ft == QH:
                                    lhsT = wk_sb[:, kt, :]
                                else:
                                    lhsT = wv_sb[:, kt, :]
                                nc.tensor.matmul(
                                    acc[:], lhsT, ht[:, kt, :],
                                    start=(kt == 0), stop=(kt == NKT - 1),
                                )
                            xb = sb.tile([128, 512], BF, tag="xb", bufs=3)
                            nc.scalar.activation(xb[:], acc[:], AF.Copy)
                            if ft < QH + 1:  # rope for q heads and k
                                rot = ps.tile([128, 512], F32, tag="mm")
                                nc.tensor.matmul(rot[:], rT[:], xb[:])
                                if ft < QH:
                                    dest = qr[
                                        :, ft * SG + sc * 512 : ft * SG + sc * 512 + 512
                                    ]
                                else:
                                    dest = kr[:, sc * 512 : sc * 512 + 512]
                                cs = c_d[:, sc * 512 : (sc + 1) * 512]
                                ss = s_d[:, sc * 512 : (sc + 1) * 512]
                                nc.vector.tensor_mul(dest, xb[:], cs)
                                rtmp = sb.tile([128, 512], BF, tag="rtmp")
                                nc.vector.tensor_mul(rtmp[:], rot[:], ss)
                                nc.vector.tensor_add(dest, dest, rtmp[:])
                            else:  # v: transpose to seq-major
                                for j in range(4):
                                    tp = ps.tile([128, 128], BF, tag="mm")
                                    nc.tensor.transpose(
                                        tp[:], xb[:, j * 128 : (j + 1) * 128], ident[:]
                                    )
                                    g = sc * 4 + j
                                    nc.vector.tensor_copy(
                                        v_seq[:, g * 128 : (g + 1) * 128], tp[:]
                                    )

            # ---- phase 2: attention (h outer so A2A can fire per head-pair)
            def attn_head(h, b, qc):
                nkt = 4 * qc + 4
                acc = psacc.tile([128, 512], F32, tag="acc")
                den = psacc.tile([1, 512], F32, tag="den")
                qs = h * SG + b * S + qc * 512

                # diagonal tile j (= kt - 4qc >= 0) only contributes to
                # q >= j*128: truncate its q range to [j*128, 512).
                def qoff(kt):
                    j = kt - 4 * qc
                    return j * 128 if j > 0 else 0

                def score(kt):
                    o = qoff(kt)
                    s_ps = ps.tile(
                        [128, 512], F32, tag="mm", name=f"s_{h}_{b}_{qc}_{kt}"
                    )
                    nc.tensor.matmul(
                        s_ps[:, : 512 - o],
                        kr[:, b * S + kt * 128 : b * S + (kt + 1) * 128],
                        qr[:, qs + o : qs + 512],
                    )
                    return s_ps

                pipe = [score(0)]
                if nkt > 1:
                    pipe.append(score(1))
                for kt in range(nkt):
                    if kt + 2 < nkt:
                        pipe.append(score(kt + 2))
                    s_ps = pipe.pop(0)
                    o = qoff(kt)
                    w = 512 - o
                    pT = sb.tile([128, 512], BF, tag="pT", bufs=3)
                    nc.scalar.activation(pT[:, :w], s_ps[:, :w], AF.Exp)
                    if kt - 4 * qc >= 0:
                        nc.vector.tensor_mul(pT[:, :w], pT[:, :w], tril[:, :w])
                    nc.tensor.matmul(
                        den[:, o:512], ones[:], pT[:, :w],
                        start=(kt == 0), stop=(kt == nkt - 1),
                    )
                    g = b * 16 + kt
                    nc.tensor.matmul(
                        acc[:, o:512], v_seq[:, g * 128 : (g + 1) * 128], pT[:, :w],
                        start=(kt == 0), stop=(kt == nkt - 1),
                    )
                recip = sb.tile([1, 512], F32, tag="recip")
                nc.vector.reciprocal_approx_fast(recip[:], den[:])
                rb = sb.tile([128, 512], F32, tag="rb")
                nc.gpsimd.partition_broadcast(rb[:], recip[:])
                ao = sb.tile([128, 512], BF, tag="ao", bufs=3)
                nc.vector.tensor_mul(ao[:], acc[:], rb[:])
                half, hh = divmod(h, 2)
                sc = b * 4 + qc
                nc.sync.dma_start(
                    a2a_in[half][sc * 256 + hh * 128 : sc * 256 + (hh + 1) * 128, :],
                    ao[:],
                )

            with nc.named_scope("attn"):
                for half in range(2):
                    for h in (2 * half, 2 * half + 1):
                        for b in range(B):
                            for qc in range(4):
                                attn_head(h, b, qc)
                    nc.gpsimd.collective_compute(
                        "AllToAll",
                        mybir.AluOpType.bypass,
                        replica_groups=[list(range(N_CORES))],
                        ins=[a2a_in[half].opt()],
                        outs=[a2a_out[half].opt()],
                    )

            # ---- phase 4: o_proj for my s-chunk, all hidden columns.
            # Two passes: pass 0 (features from A2A1) accumulates to SBUF
            # partials while A2A2 is still in flight; pass 1 adds the rest.
            with nc.named_scope("oproj"), \
                 tc.tile_pool(name="agp", bufs=1) as agp, \
                 tc.tile_pool(name="wop", bufs=3) as wop, \
                 tc.tile_pool(name="prt", bufs=1) as prt:
                parts = []
                for half in range(2):
                    agt = agp.tile([128, 16, 512], BF, tag=f"ag{half}")
                    nc.sync.dma_start(
                        agt[:],
                        a2a_out[half][:].rearrange("(ft p) s -> p ft s", p=128),
                    )
                    for hid_t in range(NKT):  # 16 tiles of 128 hidden cols
                        wo_t = wop.tile([128, 16, 128], BF, tag="wo_t")
                        nc.sync.dma_start(
                            wo_t[:],
                            wo_e[
                                half * 16 * 128 : (half * 16 + 16) * 128,
                                hid_t * 128 : (hid_t + 1) * 128,
                            ].rearrange("(ft p) c -> p ft c", p=128),
                        )
                        o_ps = ps.tile([128, 512], F32, tag="mm")
                        for ft in range(16):
                            nc.tensor.matmul(
                                o_ps[:],
                                wo_t[:, ft, :],
                                agt[:, ft, :],
                                start=(ft == 0),
                                stop=(ft == 15),
                            )
                        if half == 0:
                            part = prt.tile(
                                [128, 512], F32, tag=f"part{hid_t}"
                            )
                            nc.scalar.activation(part[:], o_ps[:], AF.Copy)
                            parts.append(part)
                        else:
                            ob = sb.tile([128, 512], F32, tag="ob")
                            nc.vector.tensor_add(ob[:], o_ps[:], parts[hid_t][:])
                            nc.sync.dma_start(
                                outT_e[hid_t * 128 : (hid_t + 1) * 128, :], ob[:]
                            )

    nc.compile()
    return nc


def _prep(hidden_states, sin_table, cos_table, Wq, Wk, Wv, Wo):
    bf = ml_dtypes.bfloat16
    flat = np.asarray(hidden_states, np.float32).reshape(SG, HID)
    hT = np.ascontiguousarray(flat.T).astype(bf)

    cosT = np.asarray(cos_table, np.float32)[:, :64].T  # [64, S]
    sinT = np.asarray(sin_table, np.float32)[:, :64].T
    c_dup = np.tile(np.concatenate([cosT, cosT], 0), (1, B)).astype(bf)
    s_dup = np.tile(np.concatenate([sinT, sinT], 0), (1, B)).astype(bf)

    R = np.zeros((D, D), np.float32)
    for i in range(64):
        R[i, i + 64] = -1.0
        R[i + 64, i] = 1.0
    rT = np.ascontiguousarray(R.T).astype(bf)
    ident = np.eye(D, dtype=np.float32).astype(bf)

    kk = np.arange(128)[:, None]
    qq = np.arange(512)[None, :]
    masks = (kk <= qq).astype(np.float32).astype(bf)

    scale = np.float32(1.0 / np.sqrt(D))
    Wq = np.asarray(Wq, np.float32) * scale
    Wk = np.asarray(Wk, np.float32)
    Wv = np.asarray(Wv, np.float32)
    Wo = np.asarray(Wo, np.float32)

    # Permute Wo rows into the order o_proj consumes the A2A output blocks:
    # a2a1 blocks: (r, h in {0,1}); a2a2 blocks: (r, h in {2,3}).
    Wo_b = Wo.reshape(H, D, HID)
    order = [4 * r + h for r in range(8) for h in (0, 1)] + [
        4 * r + h for r in range(8) for h in (2, 3)
    ]
    Wo_perm = np.ascontiguousarray(Wo_b[order].reshape(H * D, HID)).astype(bf)

    in_maps = []
    for c in range(N_CORES):
        in_maps.append(
            {
                "hT": hT,
                "wq": np.ascontiguousarray(Wq[:, c * 512 : (c + 1) * 512]).astype(bf),
                "wk": np.ascontiguousarray(Wk[:, c * D : (c + 1) * D]).astype(bf),
                "wv": np.ascontiguousarray(Wv[:, c * D : (c + 1) * D]).astype(bf),
                "wo": Wo_perm,
                "c_dup": c_dup,
                "s_dup": s_dup,
                "rT": rT,
                "ident": ident,
                "masks": masks,
            }
        )
    return in_maps


def kernel(**inputs) -> np.ndarray:
    global LAST_EXEC_NS
    if "nc" not in _CACHE:
        _CACHE["nc"] = _build()
    nc = _CACHE["nc"]

    in_maps = _prep(**inputs)
    res = run_bass_kernel_spmd(
        nc,
        in_maps,
        core_ids=list(range(N_CORES)),
        trace=bool(os.environ.get("BASS_TRACE")),
    )
    LAST_EXEC_NS = res.exec_time_ns

    outT = np.concatenate(
        [np.asarray(res.results[c]["outT"], np.float32) for c in range(N_CORES)],
        axis=1,
    )  # [HID, SG]
    return np.ascontiguousarray(outT.T).reshape(B, S, HID)
